# revision 98
# baseline (speedup 1.0000x reference)
"""Trainium2 Bass kernel for a dense transformer block (cross-attn + RoPE
self-attn + SwiGLU MLP), SPMD over 8 NeuronCores.

Sharding: core = (batch, half). Each core processes one batch (B=4) and half
its tokens (balanced causal split OWN_BLOCKS, chosen so the SPMD union of
visible key tiles is minimal). Cross-attention and self-attention K/V are
computed for the full sequence on both cores of a pair (cheap duplication —
the collective cost model prices a pair-exchange at ~15us overhead + 40GB/s,
far above the duplicated compute). Q / attention / output-proj / MLP run only
on the core's own 512 tokens.

All activations are feature-major [C, tokens]; 16-bit work runs in fp16
(e5m10 — same PE/DVE throughput as bf16, 8x the mantissa), the residual
stream itself is fp16, and matmuls accumulate in f32 PSUM. The whole SwiGLU
MLP runs in fp8(e4m3) DoubleRow (2x PE throughput): fc1 weights are prescaled
x64 (so the 0.02-scale values leave the e4m3 subnormal range) with the 1/64
in the Silu activation scale; fc2 weights are prescaled x16 so the DVE mul
writes ht = 16*h directly in fp8 range; w_proj weights (also fp8 DR) are
prescaled x64, and the combined 1/(16*64) is applied by a fused
scalar_tensor_tensor (rescale+residual add) on DVE. fc1/fc2 weights are
resident in SBUF via two big transposing DMAs issued while the SA
projections keep the PE busy; separate PSUM rings for the fc1/fc2 chains
keep the PE from parking behind the silu.

Attention is transposed-scores (S^T[k,q]) with the softmax denominator folded
into the AV matmul: V is stored [P, 16 heads, 65] with one all-ones column
per head, so PSUM rows 0:64 of the AV output are O_h and row 64 is the
denominator. exp runs unmasked on whole PSUM score groups; causal masking is
a {0,1} fp16 multiply on the exp output. Both attention loops are software-
pipelined one unit deep (scores+exp of unit i+1 are emitted before the AV
ladder of unit i) so the in-order PE queue never parks behind the Act exp
chain. Cross-attn iterates query-major with 2-tile score groups (one exp per
unit, [P,1024] 2-bank PSUM tiles) and feeds each finished 512-chunk straight
into the output projection; self-attn iterates head-major with one AV bank
and one normalize per head.

Host-side prep folds the RMSNorm gammas and the attention scale into the
weights, pre-transposes them, applies a rope-deinterleave permutation to the
self-attn q/k weights, and swizzles fm-projection weights to
[out_blk, P, in_blk, P]. x/y/wv and the CA q/k weights load via batched
transposing DMAs (the startup phase is HWDGE-issue-bound at 625ns per DMA
instruction). The even/odd-core differences (softmax masks, own-token strip
offsets) are shipped as per-core data so a single SPMD program serves all 8
cores. PSUM is re-pooled per phase (CA / SA+attention / MLP) to fit each
phase's ring layout into the 8 banks.
"""

import numpy as np
import ml_dtypes

import concourse.bacc as bacc
import concourse.bass as bass
import concourse.mybir as mybir
import concourse.tile as tile
from concourse import bass_utils
# (dynamic-AP helpers no longer needed after the token permutation)

F32 = mybir.dt.float32
# 16-bit work dtype: fp16 (e5m10) — same PE/DVE throughput as bf16 but 8x the
# mantissa. Range-safe here: scores ~N(0,0.4) so exp(score) stays < ~100, all
# weights/activations are O(1), and accumulation happens in f32 PSUM.
BF16 = mybir.dt.float16
FP8 = mybir.dt.float8e4
DR = mybir.MatmulPerfMode.DoubleRow
AF = mybir.ActivationFunctionType
ALU = mybir.AluOpType

B, T, M, C, H, FF = 4, 1024, 256, 1024, 16, 4096
HD = C // H
EPS = 1e-5
N_CORES = 8
P = 128
CT = C // P            # 8 c-tiles
TT = T // P            # 8 token blocks
T_OWN = T // 2         # 512 own tokens per core
OT = T_OWN // P        # 4 own blocks
FT = FF // P           # 32 ff tiles
MT = M // P            # 2 memory tiles (cross keys)
NEG = -1e30
WS = 64.0              # fc1 fp8 weight prescale (keeps 0.02-scale weights normal)
WS2 = 16.0             # fc2 fp8 prescale: ht = silu(h1)*(WS2*h2) = 16*h stays in range
WSP = 64.0             # w_proj fp8 prescale; proj out is (WS2*WSP)*mlp_out

# Own token blocks per half. Slot s pairs blocks (2s even-half, 2s+1-ish
# odd-half) so the SPMD union of visible key tiles per slot is minimal
# (2+4+6+8 = 20 score tiles vs 22 for other balanced splits); per-core real
# work stays causally balanced (1+4+5+8 == 2+3+6+7 == 18).
OWN_BLOCKS = {0: [0, 3, 4, 7], 1: [1, 2, 5, 6]}
# Tokens are permuted per-core so the own 512 tokens occupy cols 0:512
# (own blocks ascending), others cols 512:1024 (ascending). Self-attn slot s
# covers own col-block s; KTILES[s] is the union (over both halves) of
# visible permuted key tiles, ordered so the 2 per-half-differing tiles
# (incl. the shared-diagonal tile s) come last and get one mask multiply.
KTILES = [[4, 0],
          [0, 4, 5, 1],
          [0, 1, 4, 5, 6, 2],
          [0, 1, 2, 4, 5, 6, 7, 3]]
NMASK = 2              # masked (per-half-differing) tiles per slot

_CACHE = {}


def _bf16(a):
    return np.ascontiguousarray(a.astype(np.float16))


def _f32(a):
    return np.ascontiguousarray(a.astype(np.float32))


def _fp8(a):
    return np.ascontiguousarray(a.astype(ml_dtypes.float8_e4m3))


def _swz(wT):
    """[in, out] -> [out_blk, P, in_blk, P] so each out-block is one
    contiguous DMA into a [P, in_blk, P] SBUF tile."""
    nt, no = wT.shape[0] // P, wT.shape[1] // P
    return np.transpose(wT.reshape(nt, P, no, P), (2, 1, 0, 3))


def rope_perm():
    """Row permutation for self-attn q/k weights: per head, even hd indices
    first (rows h*32+j <- h*64+2j), all heads' real parts in rows 0:512,
    imag parts in rows 512:1024."""
    perm = np.zeros(C, dtype=np.int64)
    for h in range(H):
        for j in range(HD // 2):
            perm[h * (HD // 2) + j] = h * HD + 2 * j
            perm[C // 2 + h * (HD // 2) + j] = h * HD + 2 * j + 1
    return perm


def build_program():
    nc = bacc.Bacc("TRN2", target_bir_lowering=False, debug=False,
                   num_devices=N_CORES)

    def din(name, shape, dtype):
        return nc.dram_tensor(name, shape, dtype, kind="ExternalInput").ap()

    # x/y ship as fp16: the residual stream in fp16 costs ~2e-4 rel err
    # while halving the startup-critical x DMA
    xT = din("xT", [C, T], BF16)
    yT = din("yT", [C, M], BF16)
    # fm-projection weights are host-swizzled to [no, P, nt, P] so each
    # o-block load is one fully-contiguous DMA
    ca_wqT = din("ca_wqT", [CT, P, CT, P], BF16)
    ca_wkT = din("ca_wkT", [CT, P, CT, P], BF16)
    ca_wvT = din("ca_wvT", [C, C], BF16)
    ca_woT = din("ca_woT", [CT, P, CT, P], BF16)
    sa_wqT = din("sa_wqT", [CT, P, CT, P], BF16)
    sa_wkT = din("sa_wkT", [CT, P, CT, P], BF16)
    sa_wvT = din("sa_wvT", [C, C], BF16)
    sa_woT = din("sa_woT", [CT, P, CT, P], BF16)
    w_fc1T = din("w_fc1T", [FT, P, CT, P], FP8)
    w_fc2T = din("w_fc2T", [FT, P, CT, P], FP8)
    w_projT = din("w_projT", [CT, P, FT, P], FP8)
    cosrep = din("cosrep", [P, T], BF16)
    sinrep = din("sinrep", [P, T], BF16)
    smask = din("smask", [OT, P, NMASK * P], BF16)
    outT = nc.dram_tensor("outT", [C, T_OWN], F32, kind="ExternalOutput").ap()

    with tile.TileContext(nc) as tc:
        _body(tc, locals())
    nc.compile()
    return nc


def _body(tc, io):
    nc = tc.nc

    # ---- global pools / constants ----
    const = tc.alloc_tile_pool(name="const", bufs=1)
    ones = const.tile([P, 1], BF16)
    nc.vector.memset(ones, 1.0)
    eps_t = const.tile([1, 1], F32)
    nc.vector.memset(eps_t, EPS)
    act_pf = const.tile([1, 1], F32, tag="actpf")

    def prefetch_act(func, dep_ap):
        # dummy activation: scale=0 makes the value eps regardless of dep_ap,
        # but the read dependency places the (lazy) ACT table load right
        # after dep_ap's producer instead of on the next phase's critical
        # chain.
        nc.scalar.activation(out=act_pf, in_=dep_ap, func=func,
                             scale=0.0, bias=eps_t)
    # multiplicative {0,1} masks applied to exp(scores), [keys, queries] per
    # 128-tile: masks[s] viewed as [P, NMASK, P]
    masks = [const.tile([P, NMASK * P], BF16, tag=f"smask{s}", name=f"smask{s}")
             for s in range(OT)]
    for s in range(OT):
        nc.gpsimd.dma_start(out=masks[s], in_=io["smask"][s])

    wpool = tc.alloc_tile_pool(name="w", bufs=1)
    # PSUM is re-pooled per phase (pscur is rebound); helpers read pscur at
    # call time. CA pool: sc2_512(2x2 banks) + av(2) + acc(2) = 8 banks.
    pscur = tc.alloc_tile_pool(name="ps_ca", bufs=1, space="PSUM")
    normp = tc.alloc_tile_pool(name="normp", bufs=1)
    xopool = tc.alloc_tile_pool(name="xo", bufs=1)
    xo = [xopool.tile([P, T_OWN], BF16, tag=f"xo_{c}", name=f"xo_{c}")
          for c in range(CT)]
    attnp = tc.alloc_tile_pool(name="attnp", bufs=1)

    def wload_fused(w4, o, dtype=BF16):
        """Load the pre-swizzled o-th block of a [no, P, nt, ocols] DRAM
        weight (host layout makes this DMA fully contiguous). lhsT for
        contraction tile c is t[:, c, :]."""
        nt, ocols = w4.shape[2], w4.shape[3]
        wtb = wpool.tile([P, nt, ocols], dtype, tag=f"wtb{nt}_{ocols}_{dtype}",
                         name="wtb", bufs=4 if nt == CT and ocols == P else 3)
        nc.sync.dma_start(out=wtb, in_=w4[o])
        return wtb

    def rmsnorm(pool, src, ncols, tag, outs=None, ps_tag="acc", ps_bufs=3,
                sq_eng="mixed"):
        out = outs if outs is not None else [
            pool.tile([P, ncols], BF16, tag=f"xn_{tag}_{c}",
                      name=f"xn_{tag}_{c}") for c in range(CT)]
        for n0 in range(0, ncols, 512):
            nn = min(512, ncols - n0)
            ssq = pscur.tile([1, nn], F32, tag=ps_tag, name="ssq",
                             bufs=ps_bufs)
            for c in range(CT):
                sq = normp.tile([P, nn], BF16, tag="sq", name="sq", bufs=4)
                on_pool = (sq_eng == "pool"
                           or (sq_eng == "mixed" and c % 2 == 0))
                if on_pool:
                    nc.gpsimd.tensor_mul(out=sq, in0=src[c][:, n0:n0 + nn],
                                         in1=src[c][:, n0:n0 + nn])
                else:
                    nc.scalar.activation(out=sq, in_=src[c][:, n0:n0 + nn],
                                         func=AF.Square)
                nc.tensor.matmul(out=ssq, lhsT=ones, rhs=sq,
                                 start=(c == 0), stop=(c == CT - 1))
            # rstd = (ssq/C + eps)^-1/2 (pow is not a valid DVE ISA op, so
            # Sqrt on Act + reciprocal on DVE)
            rstd = normp.tile([1, nn], F32, tag="rstd", name="rstd", bufs=2)
            nc.scalar.activation(out=rstd, in_=ssq, func=AF.Sqrt,
                                 scale=1.0 / C, bias=eps_t)
            nc.vector.reciprocal(out=rstd, in_=rstd)
            rbc = normp.tile([P, nn], F32, tag="rbc", name="rbc", bufs=2)
            nc.gpsimd.partition_broadcast(out_ap=rbc, in_ap=rstd)
            for c in range(CT):
                nc.vector.tensor_mul(out=out[c][:, n0:n0 + nn],
                                     in0=src[c][:, n0:n0 + nn], in1=rbc)
        return out

    def proj_fm(pool, wT, xn, ncols, otiles, tag, nchunk=512, order=None,
                pbufs=3, batch4=False):
        out = [pool.tile([P, ncols], BF16, tag=f"{tag}_{o}", name=f"{tag}_{o}")
               for o in range(otiles)]

        def do_tile(o, wtb_o):
            for n0 in range(0, ncols, nchunk):
                nn = min(nchunk, ncols - n0)
                ps = pscur.tile([P, nn], F32, tag="acc", name="proj",
                                bufs=pbufs)
                for c in range(CT):
                    nc.tensor.matmul(out=ps, lhsT=wtb_o[:, c, :],
                                     rhs=xn[c][:, n0:n0 + nn],
                                     start=(c == 0), stop=(c == CT - 1))
                nc.any.tensor_copy(out=out[o][:, n0:n0 + nn], in_=ps)

        if batch4:
            # 4 o-blocks per transposing DMA: 2 HWDGE slots per weight
            # instead of 8 (the startup phase is HWDGE-issue-bound)
            for ob0 in range(0, otiles, 4):
                wtb4 = wpool.tile([P, 4, CT, P], BF16, tag="wtb4",
                                  name="wtb4", bufs=2)
                nc.sync.dma_start(
                    out=wtb4,
                    in_=wT[ob0:ob0 + 4].rearrange("o p c j -> p o c j"))
                for o in range(ob0, ob0 + 4):
                    do_tile(o, wtb4[:, o - ob0])
        else:
            for o in (order if order is not None else range(otiles)):
                do_tile(o, wload_fused(wT, o))
        return out

    def vproj_rm(pool, wT, xn, ttiles, tag, pbufs=3, dma_eng=None):
        """V in 16x65 per-head layout: out[t] is [P, 16, 65] bf16; per head
        64 v-features plus one adjacent all-ones column. The AV matmul's lhsT
        [P, 65] slice yields output partitions 0:64 = O_h and 64 = the
        softmax denominator (single free dim keeps the BIR verifier happy)."""
        out = [pool.tile([P, 16, 65], BF16, tag=f"{tag}_{t}", name=f"{tag}_{t}")
               for t in range(ttiles)]
        for t in range(ttiles):
            nc.vector.memset(out[t][:, :, 64:65], 1.0)
        for oc0 in range(0, C, 512):
            # one transposing DMA per 512-out-col group (vs 8 per-c loads):
            # the whole group is live in every t-ladder anyway
            wvb = wpool.tile([P, CT, 512], BF16, tag="wvb", name="wvb", bufs=2)
            (dma_eng or nc.sync).dma_start(
                out=wvb,
                in_=wT.rearrange("(c p) o -> p c o", p=P)[:, :, oc0:oc0 + 512])
            wts = [wvb[:, c, :] for c in range(CT)]
            for t in range(ttiles):
                ps = pscur.tile([P, 512], F32, tag="acc", name="vps",
                                bufs=pbufs)
                for c in range(CT):
                    nc.tensor.matmul(out=ps, lhsT=xn[c][:, t * P:(t + 1) * P],
                                     rhs=wts[c], start=(c == 0), stop=(c == CT - 1))
                h0 = oc0 // 64
                nc.any.tensor_copy(
                    out=out[t][:, h0:h0 + 8, 0:64],
                    in_=ps.rearrange("p (a b) -> p a b", b=64))
        return out

    def attention(pool, qT, kT, v65, q_chunks, kts, oT_tag, ncols, gsz,
                  scbufs=2, on_chunk=None):
        """Transposed-scores attention (cross-attn), denominator folded into
        the AV ladder. AV lhsT is a [P,65] view of v65 selecting {head h,
        ones}: PSUM rows 0:64 = O_h, 64 = denominator, so normalization is
        reciprocal+broadcast+mul.

        qi-major with one-unit software pipelining: scores+exp of unit i+1
        are emitted before the AV ladder of unit i, so the in-order PE queue
        never parks behind the Act exp chain. on_chunk(qi, oT) fires after a
        q-chunk's last head so the caller can fuse downstream work."""
        oT = [pool.tile([P, ncols], BF16, tag=f"{oT_tag}_{c}",
                        name=f"{oT_tag}_{c}") for c in range(CT)]
        nk = len(kts)

        def scores(h, qi):
            hp, base = h // 2, (h % 2) * 64
            q0, qn = q_chunks[qi]
            pts = []
            for g0 in range(0, nk, gsz):
                gk = min(gsz, nk - g0)
                ps = pscur.tile([P, gsz * qn], F32, tag=f"sc{gsz}_{qn}",
                                name="sc", bufs=scbufs)
                ps_r = ps.rearrange("p (a b) -> p a b", b=qn)
                for j in range(gk):
                    kt = kts[g0 + j]
                    nc.tensor.matmul(
                        out=ps_r[:, j, :],
                        lhsT=kT[hp][base:base + 64, kt * P:(kt + 1) * P],
                        rhs=qT[hp][base:base + 64, q0:q0 + qn],
                        start=True, stop=True)
                pt = attnp.tile([P, gsz, qn], BF16, tag="ptca", name="pt",
                                bufs=2)
                nc.scalar.activation(out=pt[:, 0:gk, :], in_=ps_r[:, 0:gk, :],
                                     func=AF.Exp)
                pts.append((g0, gk, pt))
            return pts

        def av(h, qi, pts):
            hp, base = h // 2, (h % 2) * 64
            q0, qn = q_chunks[qi]
            pob = pscur.tile([P, 512], F32, tag="av", name="av", bufs=2)
            po = pob[:, 0:qn]
            for g0, gk, pt in pts:
                for j in range(gk):
                    i = g0 + j
                    nc.tensor.matmul(out=po[0:65, :],
                                     lhsT=v65[kts[i]][:, h, :],
                                     rhs=pt[:, j, :],
                                     start=(i == 0), stop=(i == nk - 1))
            rcp = attnp.tile([1, 512], F32, tag="rcp", name="rcp", bufs=2)
            nc.vector.reciprocal(out=rcp[:, 0:qn], in_=pob[64:65, 0:qn])
            rbf = attnp.tile([64, 512], F32, tag="rbf", name="rbf", bufs=2)
            nc.gpsimd.partition_broadcast(out_ap=rbf[:, 0:qn],
                                          in_ap=rcp[:, 0:qn], channels=64)
            nc.vector.tensor_mul(out=oT[hp][base:base + 64, q0:q0 + qn],
                                 in0=pob[0:64, 0:qn], in1=rbf[:, 0:qn])

        # 1-unit software pipelining: scores+exp of unit i+1 are emitted
        # before the AV ladder of unit i so the in-order PE queue never parks
        # behind the Act exp chain
        units = [(h, qi) for qi in range(len(q_chunks)) for h in range(H)]
        prev = None
        for u in units:
            cur = scores(*u)
            if prev is not None:
                av(*prev)
                if prev[0] == H - 1 and on_chunk is not None:
                    on_chunk(prev[1], oT)
            prev = (*u, cur)
        av(*prev)
        if prev[0] == H - 1 and on_chunk is not None:
            on_chunk(prev[1], oT)
        return oT

    def wo_chunk(wT, oT, res_in, res_out, n0, nn, pbufs=3):
        for o in range(CT):
            wtb = wload_fused(wT, o)
            ps = pscur.tile([P, nn], F32, tag="acc", name="proj", bufs=pbufs)
            for c in range(CT):
                nc.tensor.matmul(out=ps, lhsT=wtb[:, c, :],
                                 rhs=oT[c][:, n0:n0 + nn],
                                 start=(c == 0), stop=(c == CT - 1))
            nc.vector.tensor_add(out=res_out[o][:, n0:n0 + nn], in0=ps,
                                 in1=res_in[o][:, n0:n0 + nn])

    half = CT // 2

    # ---- cross-attention (x updated in place to x') ----
    # x/y live in single big tiles so the whole residual loads in 1-2
    # transposing DMAs (the startup phase is HWDGE-issue-bound)
    xpool = tc.alloc_tile_pool(name="x", bufs=1, side="right")
    xbig = xpool.tile([P, CT, T], BF16, tag="xbig", name="xbig")
    x = [xbig[:, c, :] for c in range(CT)]
    capool = tc.alloc_tile_pool(name="ca", bufs=1, side="right")
    ybig = capool.tile([P, CT, M], BF16, tag="ybig", name="ybig")
    y = [ybig[:, c, :] for c in range(CT)]
    nc.sync.dma_start(out=ybig, in_=io["yT"].rearrange("(c p) m -> p c m", p=P))
    yn = rmsnorm(capool, y, M, "yn", ps_bufs=2)
    kca = proj_fm(capool, io["ca_wkT"], yn, M, CT, "kca", nchunk=256, pbufs=2,
                  batch4=True)
    vca = vproj_rm(capool, io["ca_wvT"], yn, MT, "vca", pbufs=2)
    xTv = io["xT"].rearrange("(c p) t -> p c t", p=P)
    for half_c in range(2):
        nc.sync.dma_start(
            out=xbig[:, :, half_c * 512:(half_c + 1) * 512],
            in_=xTv[:, :, half_c * 512:(half_c + 1) * 512])
    xn0 = rmsnorm(capool, x, T, "xn0", ps_bufs=2)
    qca = proj_fm(capool, io["ca_wqT"], xn0, T, CT, "qca", pbufs=2,
                  batch4=True)
    oca = attention(capool, qca, kca, vca, [(0, 512), (512, 512)], [0, 1],
                    "oca", T, gsz=2,
                    on_chunk=lambda qi, oT: wo_chunk(io["ca_woT"], oT,
                                                     x, x, qi * 512, 512,
                                                     pbufs=2))
    capool.release()

    # ---- self-attention (own tokens are cols 0:512 of the permuted axis) ----
    # SA/MLP PSUM pool: sc(2) + av(2) + acc(4) = 8 banks
    pscur.release()
    pscur = tc.alloc_tile_pool(name="ps_sa", bufs=1, space="PSUM")
    sa1 = tc.alloc_tile_pool(name="sa1", bufs=1)
    for c in range(CT):
        nc.vector.tensor_copy(out=xo[c], in_=x[c][:, 0:T_OWN])
    xn1 = rmsnorm(sa1, x, T, "xn1", sq_eng="act")
    xpool.release()

    xn1o = [xn1[c][:, 0:T_OWN] for c in range(CT)]

    kvq = tc.alloc_tile_pool(name="kvq", bufs=1, side="right")
    ksa = [kvq.tile([P, T], BF16, tag=f"ksa_{c}", name=f"ksa_{c}")
           for c in range(CT)]
    qsa = [kvq.tile([P, T_OWN], BF16, tag=f"qsa_{c}", name=f"qsa_{c}")
           for c in range(CT)]

    sa2 = tc.alloc_tile_pool(name="sa2", bufs=1, side="right")
    cs = sa2.tile([P, T], BF16, tag="cos", name="cs")
    sn = sa2.tile([P, T], BF16, tag="sin", name="sn")
    nc.gpsimd.dma_start(out=cs, in_=io["cosrep"])
    nc.gpsimd.dma_start(out=sn, in_=io["sinrep"])
    cso = cs[:, 0:T_OWN]
    sno = sn[:, 0:T_OWN]
    qpre = proj_fm(sa2, io["sa_wqT"], xn1o, T_OWN, CT, "qpre",
                   order=[0, 4, 1, 5, 2, 6, 3, 7])
    kpre = proj_fm(sa2, io["sa_wkT"], xn1, T, CT, "kpre",
                   order=[0, 4, 1, 5, 2, 6, 3, 7])

    def rope_rearrange(pre, cc, ss, ncols, dst):
        # pre: global-deinterleaved projection tiles; writes per-head layout
        # into dst. Pair (t, t+half) -> heads 4t..4t+3. Real/imag live in one
        # [P, 2, ncols] tile so each 64-row dst strip is a single DMA.
        for t in range(half):
            ro = sa2.tile([P, 2, ncols], BF16, tag="ro", name="ro", bufs=2)
            otr = ro[:, 0, :]
            oti = ro[:, 1, :]
            for n0 in range(0, ncols, 512):
                nn = min(512, ncols - n0)
                sl = slice(n0, n0 + nn)
                tmp = sa2.tile([P, nn], BF16, tag="ropetmp", name="ropetmp",
                               bufs=2)
                nc.vector.tensor_mul(out=otr[:, sl], in0=pre[t][:, sl],
                                     in1=cc[:, sl])
                nc.vector.tensor_mul(out=tmp, in0=pre[t + half][:, sl],
                                     in1=ss[:, sl])
                nc.vector.tensor_sub(out=otr[:, sl], in0=otr[:, sl], in1=tmp)
                tmp2 = sa2.tile([P, nn], BF16, tag="ropetmp2", name="ropetmp2",
                                bufs=2)
                nc.vector.tensor_mul(out=oti[:, sl], in0=pre[t][:, sl],
                                     in1=ss[:, sl])
                nc.vector.tensor_mul(out=tmp2, in0=pre[t + half][:, sl],
                                     in1=cc[:, sl])
                nc.vector.tensor_add(out=oti[:, sl], in0=oti[:, sl], in1=tmp2)
            for hh in range(4):
                h = 4 * t + hh
                ct, base = h // 2, (h % 2) * 64
                nc.sync.dma_start(out=dst[ct][base:base + 32, :],
                                  in_=otr[hh * 32:hh * 32 + 32, :])
                nc.sync.dma_start(out=dst[ct][base + 32:base + 64, :],
                                  in_=oti[hh * 32:hh * 32 + 32, :])

    rope_rearrange(qpre, cso, sno, T_OWN, qsa)
    rope_rearrange(kpre, cs, sn, T, ksa)
    prefetch_act(AF.Exp, ksa[CT - 1][0:1, 0:1])
    sa2.release()

    vsa = vproj_rm(kvq, io["sa_wvT"], xn1, TT, "vsa")
    sa1.release()

    # ---- fused self-attention + wo + SwiGLU MLP ----
    # MLP weights resident in SBUF (single transposing DMAs, issued while the
    # SA projections keep PE busy); proj weights stream per half.
    mlpw = tc.alloc_tile_pool(name="mlpw", bufs=1)
    wfc1 = mlpw.tile([P, FT, CT, P], FP8, tag="wfc1", name="wfc1")
    wfc2 = mlpw.tile([P, FT, CT, P], FP8, tag="wfc2", name="wfc2")
    nc.sync.dma_start(out=wfc1, in_=io["w_fc1T"].rearrange("f p c j -> p f c j"))
    nc.sync.dma_start(out=wfc2, in_=io["w_fc2T"].rearrange("f p c j -> p f c j"))
    osa = [kvq.tile([P, T_OWN], BF16, tag=f"osa_{c}", name=f"osa_{c}")
           for c in range(CT)]

    def sa_scores(h, qi):
        hp, base = h // 2, (h % 2) * 64
        kts = KTILES[qi]
        nk = len(kts)
        m0t = nk - NMASK
        q0 = qi * P
        pts = []
        for g0 in range(0, nk, 4):
            gk = min(4, nk - g0)
            ps = pscur.tile([P, 512], F32, tag="sc", name="sc", bufs=3)
            ps_r = ps.rearrange("p (a b) -> p a b", b=P)
            for j in range(gk):
                kt = kts[g0 + j]
                nc.tensor.matmul(
                    out=ps_r[:, j, :],
                    lhsT=ksa[hp][base:base + 64, kt * P:(kt + 1) * P],
                    rhs=qsa[hp][base:base + 64, q0:q0 + P],
                    start=True, stop=True)
            pt = attnp.tile([P, 4, P], BF16, tag="pt", name="pt", bufs=6)
            nc.scalar.activation(out=pt[:, 0:gk, :], in_=ps_r[:, 0:gk, :],
                                 func=AF.Exp)
            lo, hi = max(m0t, g0), min(nk, g0 + gk)
            if lo < hi:
                mview = masks[qi].rearrange("p (a b) -> p a b", b=P)
                nc.vector.tensor_mul(out=pt[:, lo - g0:hi - g0, :],
                                     in0=pt[:, lo - g0:hi - g0, :],
                                     in1=mview[:, lo - m0t:hi - m0t, :])
            pts.append((g0, gk, pt))
        return pts

    avb = {}

    def sa_av(h, qi, pts):
        hp, base = h // 2, (h % 2) * 64
        kts = KTILES[qi]
        nk = len(kts)
        # per-head AV bank spanning all 4 q-chunks; one normalize per head
        if qi == 0:
            avb[h] = pscur.tile([P, 512], F32, tag="av", name="av", bufs=2)
        pob = avb[h]
        po = pob.rearrange("p (a b) -> p a b", b=P)[:, qi, :]
        for g0, gk, pt in pts:
            for j in range(gk):
                i = g0 + j
                nc.tensor.matmul(out=po[0:65, :], lhsT=vsa[kts[i]][:, h, :],
                                 rhs=pt[:, j, :], start=(i == 0),
                                 stop=(i == nk - 1))
        if qi == OT - 1:
            rcp = attnp.tile([1, 512], F32, tag="rcp", name="rcp", bufs=2)
            nc.vector.reciprocal(out=rcp, in_=pob[64:65, :])
            rbf = attnp.tile([64, 512], F32, tag="rbf", name="rbf", bufs=2)
            nc.gpsimd.partition_broadcast(out_ap=rbf, in_ap=rcp, channels=64)
            nc.vector.tensor_mul(out=osa[hp][base:base + 64, :],
                                 in0=pob[0:64, :], in1=rbf)

    # h-major with 1-unit software pipelining (scores of unit i+1 before the
    # AV ladder of unit i)
    units = [(h, qi) for h in range(H) for qi in range(OT)]
    prev = None
    for u in units:
        cur = sa_scores(*u)
        if prev is not None:
            sa_av(prev[0], prev[1], prev[2])
        prev = (u[0], u[1], cur)
    sa_av(prev[0], prev[1], prev[2])

    # ---- wo + SwiGLU MLP on own tokens, full-width, fully fp8 DoubleRow ----
    # separate PSUM rings for the fc1 (gate) and fc2 chains so the PE never
    # parks behind the silu: z(f) is freed by the silu, h2(f) by the mul
    wo_chunk(io["sa_woT"], osa, xo, xo, 0, T_OWN)
    kvq.release()
    pscur.release()
    pscur = tc.alloc_tile_pool(name="ps_mlp", bufs=1, space="PSUM")
    mpool = tc.alloc_tile_pool(name="mlp", bufs=1)
    xn2f = mpool.tile([P, CT, T_OWN], FP8, tag="xn2f", name="xn2f")
    rmsnorm(mpool, xo, T_OWN, "xn2",
            outs=[xn2f[:, c, :] for c in range(CT)],
            ps_tag="nrm", ps_bufs=1)
    htf = mpool.tile([P, FT, T_OWN], FP8, tag="htf", name="htf")
    for f in range(FT):
        z = pscur.tile([P, T_OWN], F32, tag="fc1", name="fc1", bufs=2)
        for cc in range(CT // 2):
            nc.tensor.matmul(out=z, lhsT=wfc1[:, f, 2 * cc:2 * cc + 2, :],
                             rhs=xn2f[:, 2 * cc:2 * cc + 2, :],
                             start=(cc == 0), stop=(cc == CT // 2 - 1),
                             perf_mode=DR)
        s1t = mpool.tile([P, T_OWN], BF16, tag="silu", name="silu", bufs=3)
        # z = WS*h1, so silu is evaluated at scale 1/WS
        nc.scalar.activation(out=s1t, in_=z, func=AF.Silu, scale=1.0 / WS)
        h2 = pscur.tile([P, T_OWN], F32, tag="fc2", name="fc2", bufs=2)
        for cc in range(CT // 2):
            nc.tensor.matmul(out=h2, lhsT=wfc2[:, f, 2 * cc:2 * cc + 2, :],
                             rhs=xn2f[:, 2 * cc:2 * cc + 2, :],
                             start=(cc == 0), stop=(cc == CT // 2 - 1),
                             perf_mode=DR)
        # htf = silu(h1) * (WS2*h2) = WS2*h, comfortably inside e4m3 range
        nc.vector.tensor_mul(out=htf[:, f, :], in0=s1t, in1=h2)
    for o in range(CT):
        wpb = wload_fused(io["w_projT"], o, dtype=FP8)
        ps = pscur.tile([P, T_OWN], F32, tag="prj", name="proj", bufs=3)
        for g in range(FT // 2):
            nc.tensor.matmul(out=ps, lhsT=wpb[:, 2 * g:2 * g + 2, :],
                             rhs=htf[:, 2 * g:2 * g + 2, :],
                             start=(g == 0), stop=(g == FT // 2 - 1),
                             perf_mode=DR)
        # ps = (WS2*WSP) * mlp_out; fused rescale + residual add on DVE
        ob = mpool.tile([P, T_OWN], F32, tag="ob", name="ob", bufs=2)
        nc.vector.scalar_tensor_tensor(
            out=ob, in0=ps, scalar=1.0 / (WS2 * WSP), in1=xo[o],
            op0=ALU.mult, op1=ALU.add)
        nc.sync.dma_start(out=io["outT"][o * P:(o + 1) * P, :], in_=ob)
    mpool.release()
    mlpw.release()
    attnp.release()
    xopool.release()
    normp.release()
    wpool.release()
    pscur.release()
    const.release()


def prep_inputs(inputs):
    """Host-side prep: transpose/permute/fold weights, build per-core maps."""
    g = {k: np.asarray(v) for k, v in inputs.items()}
    scale = 1.0 / np.sqrt(HD)
    g0, g0t, g1, g2 = g["ln0_s"], g["ln0t_s"], g["ln1_s"], g["ln2_s"]
    perm = rope_perm()

    shared = {
        "ca_wqT": _bf16(_swz(((g["ca_wq"] * scale) * g0[None, :]).T)),
        "ca_wkT": _bf16(_swz((g["ca_wk"] * g0t[None, :]).T)),
        "ca_wvT": _bf16((g["ca_wv"] * g0t[None, :]).T),
        "ca_woT": _bf16(_swz(g["ca_wo"].T)),
        "sa_wqT": _bf16(_swz(((g["sa_wq"] * scale) * g1[None, :])[perm, :].T)),
        "sa_wkT": _bf16(_swz((g["sa_wk"] * g1[None, :])[perm, :].T)),
        "sa_wvT": _bf16((g["sa_wv"] * g1[None, :]).T),
        "sa_woT": _bf16(_swz(g["sa_wo"].T)),
        "w_fc1T": _fp8(_swz((g["w_fc1"] * g2[None, :] * WS).T)),
        "w_fc2T": _fp8(_swz((g["w_fc2"] * g2[None, :] * WS2).T)),
        "w_projT": _fp8(_swz((g["w_proj"] * WSP).T)),
    }
    cosT = _f32(g["cos"].T)   # [32, T]
    sinT = _f32(g["sin"].T)
    cosrep = np.tile(cosT, (4, 1))
    sinrep = np.tile(sinT, (4, 1))

    for nm in ["ca_bq", "ca_bk", "ca_bv", "ca_bo",
               "sa_bq", "sa_bk", "sa_bv", "sa_bo"]:
        assert not np.any(g[nm]), f"nonzero bias {nm} unsupported"
    assert bool(np.all(g["padding_mask"])), "padding_mask must be all ones"

    # per-half token permutation: own blocks (ascending) to cols 0:512,
    # the other half's tokens (ascending) to cols 512:1024
    tperm = {}
    for h in (0, 1):
        own = np.concatenate([np.arange(qt * P, (qt + 1) * P)
                              for qt in OWN_BLOCKS[h]])
        other = np.setdiff1d(np.arange(T), own)
        tperm[h] = np.concatenate([own, other])

    # per-half multiplicative masks on exp(scores), over the permuted key
    # axis: slot s (own col-block s, diag tile s) iterates KTILES[s]; the
    # last NMASK entries are the per-half-differing tiles shipped as {0,1}
    # bf16 data ([keys, queries] orientation, diag = triu).
    masks = {}
    for h in (0, 1):
        sm = np.ones((OT, P, NMASK * P), np.float32)
        for s in range(OT):
            q_tok = tperm[h][s * P:(s + 1) * P]          # global token ids
            for j, kt in enumerate(KTILES[s][-NMASK:]):
                k_tok = tperm[h][kt * P:(kt + 1) * P]
                sm[s, :, j * P:(j + 1) * P] = (
                    k_tok[:, None] <= q_tok[None, :]).astype(np.float32)
        masks[h] = _bf16(sm)

    x, y = _f32(g["x"]), _f32(g["y"])
    in_maps = []
    for core in range(N_CORES):
        b, h = core // 2, core % 2
        m = dict(shared)
        m["xT"] = _bf16(x[b].T[:, tperm[h]])
        m["yT"] = _bf16(y[b].T)
        m["cosrep"] = _bf16(cosrep[:, tperm[h]])
        m["sinrep"] = _bf16(sinrep[:, tperm[h]])
        m["smask"] = masks[h]
        in_maps.append(m)
    return in_maps


def assemble_output(results, dtype):
    out = np.empty((B, T, C), np.float32)
    for core in range(N_CORES):
        b, h = core // 2, core % 2
        oT = results[core]["outT"]          # [C, T_OWN]
        for li, qt in enumerate(OWN_BLOCKS[h]):
            out[b, qt * P:(qt + 1) * P, :] = oT[:, li * P:(li + 1) * P].T
    return out.astype(dtype, copy=False)


def kernel(**inputs):
    if "nc" not in _CACHE:
        _CACHE["nc"] = build_program()
    nc = _CACHE["nc"]
    in_maps = prep_inputs(inputs)
    res = bass_utils.run_bass_kernel_spmd(nc, in_maps,
                                          core_ids=list(range(N_CORES)))
    return assemble_output(res.results, np.asarray(inputs["x"]).dtype)

